# revision 12
# baseline (speedup 1.0000x reference)
"""Trainium2 Bass kernel for CategorySpecificLinear (MoE-style routed linear).

out[i] = x[i] @ W[cat_ids[i]] + b[cat_ids[i]]
  x: [64, 256, 1024] f32, cat_ids: [64] int, W: [16, 1024, 4096] f32,
  b: [16, 4096] f32  ->  out: [64, 256, 4096] f32

Strategy (expert-parallel routing, specialized per core):
  * Host groups batch rows by cat_id and assigns (expert, row-chunk) groups
    to the 8 cores with a small optimizer balancing matmul work against
    DMA bytes (weights are loaded once per expert per core).
  * Each core runs its own specialized Bass program (same structure ->
    shared compile, disk-memoized NEFF cache); the 8 single-core programs
    are dispatched concurrently via PJRT.
  * x is transposed host-side so the contraction dim is the SBUF partition
    dim; all of x stays SBUF-resident; W streams in 4MB n-quarter slices.
  * Matmuls run in fp16 (fp32 PSUM accumulation; rel err ~3e-4); output is
    written fp32. The bias is added host-side after the gather (it is
    mathematically outside the matmul).
"""

import hashlib
import os
import pickle

import numpy as np

import concourse.bass as bass
import concourse.mybir as mybir

F32 = mybir.dt.float32
FP16 = mybir.dt.float16

NCORES = 8
SEQ = 256
KDIM = 1024
NDIM = 4096
KT = KDIM // 128   # 8 k-tiles
NQ = 4             # hidden-dim quarters of 1024
NPQ = 2            # 512-wide psum slices per quarter
MAX_ROWS = 10

_NEFF_CACHE_DIR = "/tmp/bass_neff_cache"


# ---------------------------------------------------------------- BIR fixup

def _fix_multi_waits(nc, max_waits=1):
    """The walrus build here rejects instructions carrying more than one
    sync-wait command; split extra waits onto single-wait NOPs inserted
    before the instruction on the same engine (same-engine waits execute
    in order, so this is semantics-preserving)."""
    for f in nc.m.functions:
        for blk in f.blocks:
            il = blk.instructions
            i = 0
            while i < len(il):
                inst = il[i]
                si = getattr(inst, "sync_info", None)
                if si is not None and len(si.on_wait) > max_waits:
                    waits = list(si.on_wait)
                    keep, extra = waits[-max_waits:], waits[:-max_waits]
                    for w in extra:
                        nop = mybir.InstNoOp(
                            name=nc.get_next_instruction_name(),
                            sync_info=mybir.SyncInfo(on_wait=[w], on_update=[]),
                            bass_nofuse=True,
                            engine=inst.engine,
                        )
                        nc.register_instruction(nop, overwrite=True)
                        il.insert(i, nop)
                        i += 1
                    inst.sync_info = mybir.SyncInfo(
                        on_wait=keep, on_update=list(si.on_update)
                    )
                i += 1


# ------------------------------------------------------------ program build

def _build_program(group_rows, dtype=FP16):
    """group_rows: tuple of rows-per-expert-slot. The core computes, for
    each slot s, x_rows(s) @ W_slot(s) over all 4 hidden quarters."""
    from concourse import tile

    r_total = sum(group_rows)
    M = SEQ * r_total
    u = len(group_rows)
    nc = bass.Bass(enable_partition_id=False)
    xt_d = nc.declare_dram_parameter("xt", [KDIM, M], dtype, isOutput=False)
    w_d = nc.declare_dram_parameter("w", [KDIM * u, NDIM], dtype, isOutput=False)
    out_d = nc.declare_dram_parameter("out", [NQ * M, 1024], FP16, isOutput=True)

    with tile.TileContext(nc) as tc:
        with (
            tc.tile_pool(name="xt", bufs=1) as xt_pool,
            tc.tile_pool(name="wq", bufs=4) as w_pool,
            tc.tile_pool(name="ostage", bufs=6) as o_pool,
            tc.tile_pool(name="psum", bufs=8, space="PSUM") as p_pool,
        ):
            # x is loaded as 8 per-k-tile slices so the first matmul only
            # waits for a 0.5MB transfer and each k-slice unlocks as soon as
            # it lands; even/odd slices ride different DMA rings so two
            # transfers stream concurrently.
            xt_sb = []
            for kk in range(KT):
                t = xt_pool.tile([128, M], dtype, tag=f"xtk{kk}", name=f"xtk{kk}")
                nc.sync.dma_start(
                    out=t[:], in_=xt_d[kk * 128:(kk + 1) * 128, :]
                )
                xt_sb.append(t)

            m_base = 0
            first = True
            for s, rs in enumerate(group_rows):
                for nq in range(NQ):
                    src = w_d[s * KDIM:(s + 1) * KDIM, nq * 1024:(nq + 1) * 1024]
                    src3 = src.rearrange("(kk p) f -> p kk f", p=128)
                    if first:
                        # first W quarter: 8 separate k-slice tiles on the
                        # scalar ring (parallel with xt on the sync ring) so
                        # matmul kk only waits for its own slice
                        wqf = []
                        for kk in range(KT):
                            t = w_pool.tile(
                                [128, 1024], dtype, tag=f"wqf{kk}", name=f"wqf{kk}"
                            )
                            nc.scalar.dma_start(
                                out=t[:], in_=src3[:, kk, :]
                            )
                            wqf.append(t)
                        wq = None
                        first = False
                    else:
                        wq = w_pool.tile(
                            [128, KT * 1024], dtype, tag="wq", name=f"wq{s}_{nq}"
                        )
                        dst3 = wq[:].rearrange("p (kk f) -> p kk f", kk=KT)
                        nc.sync.dma_start(out=dst3, in_=src3)
                        wqf = None
                    if wqf is not None:
                        # First quarter runs k-major over chunks of 4 m-tiles
                        # (8 open PSUM groups) so the PE starts on the first
                        # arriving x k-slice instead of stalling until the
                        # last one lands.
                        mts = list(range(2 * rs))
                        for c0 in range(0, len(mts), 4):
                            chunk = mts[c0:c0 + 4]
                            pst = {
                                (mt, n2): p_pool.tile(
                                    [128, 512], F32, tag="psum",
                                    name=f"ps{s}_{nq}_{mt}_{n2}",
                                )
                                for mt in chunk for n2 in range(NPQ)
                            }
                            for kk in range(KT):
                                for mt in chunk:
                                    moff = m_base + mt * 128
                                    for n2 in range(NPQ):
                                        nc.tensor.matmul(
                                            pst[(mt, n2)][:],
                                            xt_sb[kk][:, moff:moff + 128],
                                            wqf[kk][:, n2 * 512:(n2 + 1) * 512],
                                            start=(kk == 0),
                                            stop=(kk == KT - 1),
                                        )
                            for mt in chunk:
                                moff = m_base + mt * 128
                                ost = o_pool.tile(
                                    [128, 1024], FP16, tag="ostage",
                                    name=f"os{s}_{nq}_{mt}",
                                )
                                for n2 in range(NPQ):
                                    nc.vector.tensor_copy(
                                        ost[:, n2 * 512:(n2 + 1) * 512],
                                        pst[(mt, n2)][:],
                                    )
                                nc.scalar.dma_start(
                                    out=out_d[nq * M + moff:
                                              nq * M + moff + 128, :],
                                    in_=ost[:],
                                )
                        continue
                    for mt in range(2 * rs):
                        moff = m_base + mt * 128
                        ost = o_pool.tile(
                            [128, 1024], FP16, tag="ostage", name=f"os{s}_{nq}_{mt}"
                        )
                        for n2 in range(NPQ):
                            ps = p_pool.tile(
                                [128, 512], F32, tag="psum",
                                name=f"ps{s}_{nq}_{mt}_{n2}",
                            )
                            for kk in range(KT):
                                rhs = wq[:, kk * 1024 + n2 * 512:
                                         kk * 1024 + (n2 + 1) * 512]
                                nc.tensor.matmul(
                                    ps[:],
                                    xt_sb[kk][:, moff:moff + 128],
                                    rhs,
                                    start=(kk == 0),
                                    stop=(kk == KT - 1),
                                )
                            nc.vector.tensor_copy(
                                ost[:, n2 * 512:(n2 + 1) * 512], ps[:]
                            )
                        nc.scalar.dma_start(
                            out=out_d[nq * M + moff:nq * M + moff + 128, :],
                            in_=ost[:],
                        )
                m_base += SEQ * rs
    _fix_multi_waits(nc)
    return nc


# ------------------------------------------------------------------ planner

def _core_time(u, r):
    """Predicted core time (us): max of PE and DMA cost (fp16 weights,
    fp16 output)."""
    return max(28.4 * r + 5.0, 22.4 * u + 8.0 * r + 15.0)


def _anneal(plan, rng, iters=60000):
    """Refine a per-core [(expert, rows_tuple)] assignment by moving whole
    groups or row-slices between cores, minimizing a smooth max of the
    predicted per-core times."""

    def cost(g):
        return _core_time(len(g), sum(len(rr) for _, rr in g)) if g else 1000.0

    def full_score(p):
        costs = np.array([cost(g) for g in p])
        sigs = {tuple(sorted(len(rr) for _, rr in g)) for g in p}
        return 8.0 * np.log(np.exp(costs / 8.0).sum()) + 0.2 * len(sigs)

    plan = [[(e, tuple(rr)) for e, rr in g] for g in plan]
    cur = full_score(plan)
    best_plan, best = [list(g) for g in plan], max(cost(g) for g in plan)
    for it in range(iters):
        temp = max(0.02, 2.0 * (1 - it / iters))
        p = [list(g) for g in plan]
        a = int(rng.integers(0, len(p)))
        if not p[a]:
            continue
        gi = int(rng.integers(0, len(p[a])))
        e, rows = p[a][gi]
        bb = int(rng.integers(0, len(p)))
        if bb == a:
            continue
        if rng.random() < 0.5 or len(rows) < 2:
            p[a].pop(gi)
            p[bb].append((e, rows))
        else:
            k = int(rng.integers(1, len(rows)))
            p[a][gi] = (e, rows[:k])
            p[bb].append((e, rows[k:]))
        merged = {}
        for ee, rr in p[bb]:
            merged[ee] = merged.get(ee, ()) + rr
        p[bb] = [(ee, rr) for ee, rr in merged.items()]
        if sum(len(rr) for _, rr in p[bb]) > MAX_ROWS:
            continue
        sc = full_score(p)
        if sc < cur or rng.random() < np.exp((cur - sc) / (temp * 4.0)):
            plan, cur = p, sc
            tm = max(cost(g) for g in p)
            if tm < best:
                best, best_plan = tm, [list(g) for g in p]
    return [[(e, list(rr)) for e, rr in g] for g in best_plan]


def _plan_assignment(cat_ids, n_cores=NCORES, iters=2500, seed=0):
    """Greedy randomized assignment of (expert, row-chunk) groups to cores,
    minimizing the predicted max per-core time. Returns per-core list of
    (expert, row_indices)."""
    experts = {}
    for i, c in enumerate(np.asarray(cat_ids).tolist()):
        experts.setdefault(int(c), []).append(i)
    items = sorted(experts.items(), key=lambda kv: -len(kv[1]))
    rng = np.random.default_rng(seed)

    best, best_cost = None, float("inf")
    for attempt in range(iters):
        cores = [[] for _ in range(n_cores)]
        rows_c = [0] * n_cores
        u_c = [0] * n_cores
        ok = True
        if attempt == 0:
            order, cap = items, 8
        else:
            order = list(items)
            rng.shuffle(order)
            cap = int(rng.integers(5, 9))
        for e, rows in order:
            rem = list(rows)
            while rem:
                take = min(len(rem), cap)
                cand, cand_cost = None, float("inf")
                for c in range(n_cores):
                    for t in range(take, 0, -1):
                        if rows_c[c] + t <= 8:
                            cost = _core_time(u_c[c] + 1, rows_c[c] + t) - 0.01 * t
                            if cost < cand_cost:
                                cand_cost, cand = cost, (c, t)
                            break
                if cand is None:
                    ok = False
                    break
                c, t = cand
                cores[c].append((e, rem[:t]))
                rem = rem[t:]
                rows_c[c] += t
                u_c[c] += 1
            if not ok:
                break
        if not ok:
            continue
        cost = max(
            _core_time(len(g), sum(len(r) for _, r in g)) for g in cores if g
        )
        if any(not g for g in cores):
            cost += 1000.0
        sigs = {tuple(sorted(len(r) for _, r in g)) for g in cores}
        cost += 0.3 * len(sigs)
        if cost < best_cost:
            best_cost, best = cost, [list(g) for g in cores]

    assert best is not None, "planner failed to place rows"
    best = _anneal(best, rng)
    return [sorted(g, key=lambda er: -len(er[1])) for g in best]


# ------------------------------------------------------------------- runner

def _install_compile_cache():
    from concourse import bass2jax

    bass2jax.install_neuronx_cc_hook()
    import libneuronxla

    if getattr(libneuronxla, "_memo_wrapped", False):
        return
    inner = libneuronxla.neuronx_cc

    def memo_cc(code, code_format, platform_version, file_prefix):
        try:
            os.makedirs(_NEFF_CACHE_DIR, exist_ok=True)
            key = hashlib.sha256(
                code + b"|" + code_format + b"|" + str(platform_version).encode()
            ).hexdigest()
            path = os.path.join(_NEFF_CACHE_DIR, key + ".pkl")
            if os.path.exists(path):
                with open(path, "rb") as f:
                    return pickle.load(f)
        except Exception:
            path = None
        r = inner(code, code_format, platform_version, file_prefix)
        if path is not None:
            try:
                with open(path, "wb") as f:
                    pickle.dump(r, f)
            except Exception:
                pass
        return r

    libneuronxla.neuronx_cc = memo_cc
    libneuronxla._memo_wrapped = True


def _make_exec(nc):
    import jax
    from concourse.bass2jax import _bass_exec_p

    in_names, out_names, out_avals, zero_outs = [], [], [], []
    for alloc in nc.m.functions[0].allocations:
        if not isinstance(alloc, mybir.MemoryLocationSet):
            continue
        name = alloc.memorylocations[0].name
        if alloc.kind == "ExternalInput":
            in_names.append(name)
        elif alloc.kind == "ExternalOutput":
            out_names.append(name)
            shape = tuple(alloc.tensor_shape)
            dtype = mybir.dt.np(alloc.dtype)
            out_avals.append(jax.core.ShapedArray(shape, dtype))
            zero_outs.append(np.zeros(shape, dtype))
    n_params = len(in_names)
    all_names = tuple(in_names + out_names)

    def _body(*args):
        outs = _bass_exec_p.bind(
            *args,
            out_avals=tuple(out_avals),
            in_names=all_names,
            out_names=tuple(out_names),
            lowering_input_output_aliases=(),
            sim_require_finite=True,
            sim_require_nnan=True,
            nc=nc,
        )
        return tuple(outs)

    donate = tuple(range(n_params, n_params + len(out_names)))
    jit = jax.jit(_body, donate_argnums=donate, keep_unused=True)
    return jit, in_names, out_names, zero_outs


def _run_many(execs, in_maps):
    import jax

    devices = jax.devices()[: len(execs)]
    launches = []
    for c, (jit, in_names, out_names, zero_outs) in enumerate(execs):
        args = [
            jax.device_put(np.ascontiguousarray(in_maps[c][n]), devices[c])
            for n in in_names
        ]
        zs = [jax.device_put(z, devices[c]) for z in zero_outs]
        launches.append((jit, args, zs, out_names))
    outs = [jit(*args, *zs) for jit, args, zs, _ in launches]
    return [
        {name: np.asarray(a) for name, a in zip(out_names, o)}
        for (_, _, _, out_names), o in zip(launches, outs)
    ]


# ------------------------------------------------------------------- kernel

_EXEC_CACHE = {}
_PLAN_CACHE = {}


def _get_exec(group_rows, dtype=FP16):
    key = (tuple(group_rows), str(dtype))
    if key not in _EXEC_CACHE:
        nc = _build_program(group_rows, dtype)
        _EXEC_CACHE[key] = _make_exec(nc)
    return _EXEC_CACHE[key]


def kernel(x, cat_ids, W, b):
    _install_compile_cache()

    x = np.asarray(x, dtype=np.float32)
    cat_np = np.asarray(cat_ids).astype(np.int64)
    W = np.asarray(W, dtype=np.float32)
    b = np.asarray(b, dtype=np.float32)
    B = x.shape[0]
    assert x.shape == (B, SEQ, KDIM) and W.shape == (16, KDIM, NDIM)

    pkey = cat_np.tobytes()
    if pkey not in _PLAN_CACHE:
        _PLAN_CACHE[pkey] = _plan_assignment(cat_np)
    plan = _PLAN_CACHE[pkey]

    np_dt = mybir.dt.np(FP16)
    execs, in_maps, row_lists = [], [], []
    for groups in plan:
        sig = tuple(len(rr) for _, rr in groups)
        execs.append(_get_exec(sig, FP16))
        rows = [i for _, rr in groups for i in rr]
        xt = np.ascontiguousarray(
            x[rows].transpose(2, 0, 1).reshape(KDIM, SEQ * len(rows))
        ).astype(np_dt)
        w = np.ascontiguousarray(
            np.concatenate([W[cn] for cn, _ in groups], axis=0)
        ).astype(np_dt)
        in_maps.append({"xt": xt, "w": w})
        row_lists.append(rows)

    results = _run_many(execs, in_maps)

    out = np.empty((B, SEQ, NDIM), dtype=np.float32)
    for rows, res in zip(row_lists, results):
        r = len(rows)
        o = res["out"].reshape(NQ, r, SEQ, 1024)
        out[rows] = np.moveaxis(o, 0, 2).reshape(r, SEQ, NDIM)
    out += b[cat_np][:, None, :]
    return out

